# revision 26
# baseline (speedup 1.0000x reference)
"""Trainium2 Bass kernel for e3nn-style BatchNorm (instance norm over graphs).

Problem: x [200000, 480] f32, irreps 128x0e + 64x1o + 32x2e, batch_id sorted
into 64 graphs, weight [224], bias [128].

Math (per graph g, derived from the reference):
  scalar block (cols 0:128, one col per channel c):
    m[g,c]   = mean_g(x_c)
    var[g,c] = mean_g(x_c^2) - m^2
    A[g,c]   = w_c / sqrt(var + eps);  B[g,c] = bias_c - m*A
    out      = x*A + B
  vector blocks (64 chans x dim 3, 32 chans x dim 5):
    fn[g,j]  = mean_g(mean_d(x^2))  = (1/d) * sum_d mean_g(x_jd^2)
    A[g,j]   = w_j / sqrt(fn + eps);  out = x*A

Sharding: 8 graphs per core (graph-aligned boundaries via searchsorted on the
host), each core's rows padded to a common N_pad.  All stats are local to a
core -> no collectives.

Single-pass sliding window: batch_id is sorted, so once the stats pass has
consumed the last row of every graph touching group t, group t (still
resident in SBUF) can be normalized and written out -- x is read once.  The
group -> ready-point map f*(t) is data-dependent; the host computes it
(max across cores, so one SPMD program serves all 8) and the program is
specialized to it (compiled inside kernel(), cached by (n_pad, f*)).  Falls
back to a two-pass program when the needed window exceeds SBUF.

Per group: a ones column rides at col 480 (local graph id at col 481) so the
481-wide fp16 squared matmul against the per-row one-hot also produces the
per-graph counts; a second skinny fp16 matmul over an fp16 copy of cols
0:128 gives sum-of-x.  At each ready point the per-graph affine params
(A | B) are (re)computed from the accumulated sums -- entries for incomplete
graphs are garbage but sanitized to stay finite, and no ready row refers to
them.  The apply gathers params to rows with two fp16 matmuls (hi/lo split,
fp32 PSUM accumulate -> ~1e-5 accurate), batched so 4 wide DVE ops cover
HGRP subtiles, then writes out in-place.

Rows are mapped so each partition holds RPP=4 consecutive rows per slot ->
7.7KB contiguous DMA descriptors (near line-rate HBM).  fp16 single-pass
matmuls are ~4x cheaper on the PE than fp32 LOW_HIGH; fp16 stats noise is
~3e-5 relative.
"""

import sys

if "/opt/trn_rl_repo" not in sys.path:
    sys.path.insert(0, "/opt/trn_rl_repo")

import numpy as np

P = 128          # partitions / rows per subtile
KS = 2           # k-chunks per group
RPP = 4          # consecutive rows per (partition, k) slot -> 7.7KB DMA chunks
KSUB = KS * RPP  # 128-row subtiles per group
GROUP = P * KSUB # rows per group (1024)
C = 480          # data columns
CW = C + 2       # + ones col (C) + local graph id col (C+1)
NCORES = 8
G = 64           # total graphs
GPC = G // NCORES  # graphs per core
EPS = 1e-5
HGRP = 2         # subtiles whose gathered params share one PSUM tile
W_MAX = 9        # sliding-window tiles (stream path)
R_CACHE = 6      # two-pass fallback: trailing groups kept in SBUF
XT_BUFS = 4

_prog_cache = {}


def _expand(ap_in, rep, bass):
    """Append a trailing broadcast dim [0, rep] to an AP."""
    return bass.AP(tensor=ap_in.tensor, offset=ap_in.offset,
                   ap=[*ap_in.ap, [0, rep]])


def _setup(nc, bass, tile, mybir, tc, cp):
    """Constant tiles shared by both builders."""
    f32 = mybir.dt.float32
    consts = {}
    iota_t = cp.tile([P, GPC], f32, tag="iota_t")
    nc.gpsimd.dma_start(out=iota_t[:], in_=bass.AP(
        tensor=nc.t_iota, offset=0, ap=[[0, P], [1, GPC]]))
    iota_c = cp.tile([GPC, 1], f32, tag="iota_c")
    nc.gpsimd.dma_start(out=iota_c[:], in_=bass.AP(
        tensor=nc.t_iota, offset=0, ap=[[1, GPC], [1, 1]]))
    w_b = cp.tile([GPC, 224], f32, tag="w_b")
    nc.gpsimd.dma_start(out=w_b[:], in_=bass.AP(
        tensor=nc.t_w, offset=0, ap=[[0, GPC], [1, 224]]))
    bias_b = cp.tile([GPC, 128], f32, tag="bias_b")
    nc.gpsimd.dma_start(out=bias_b[:], in_=bass.AP(
        tensor=nc.t_b, offset=0, ap=[[0, GPC], [1, 128]]))
    eps_t = cp.tile([GPC, 1], f32, tag="eps_t")
    nc.vector.memset(eps_t[:], EPS)
    consts.update(iota_t=iota_t, iota_c=iota_c, w_b=w_b, bias_b=bias_b,
                  eps_t=eps_t)
    return consts


def _declare_io(nc, mybir, n_pad):
    f32 = mybir.dt.float32
    nc.t_x = nc.dram_tensor("x", [n_pad, CW], f32, kind="ExternalInput")
    nc.t_bid = nc.dram_tensor("bid", [n_pad], f32, kind="ExternalInput")
    nc.t_iota = nc.dram_tensor("iota8", [GPC], f32, kind="ExternalInput")
    nc.t_w = nc.dram_tensor("w", [224], f32, kind="ExternalInput")
    nc.t_b = nc.dram_tensor("b", [128], f32, kind="ExternalInput")
    nc.t_out = nc.dram_tensor("out", [n_pad, C], f32, kind="ExternalOutput")
    # row (g*GROUP + k*P*RPP + p*RPP + r) -> [g][p][k][r]: each partition
    # holds RPP consecutive rows per k slot (contiguous DMA chunks)
    x_g = nc.t_x.ap().rearrange("(g k p r) c -> g p k r c", p=P, k=KS, r=RPP)
    out_g = nc.t_out.ap().rearrange("(g k p r) c -> g p k r c",
                                    p=P, k=KS, r=RPP)
    return x_g, out_g


def _phase1_ops(nc, bass, mybir, sqp, ohp, xt, cs):
    """Square/copy/one-hot for one group; returns (sq, xbf, oh)."""
    bf16 = mybir.dt.float16
    Act = mybir.ActivationFunctionType
    Alu = mybir.AluOpType
    sq = sqp.tile([P, KS, RPP, C + 1], bf16, tag="sq")
    nc.scalar.activation(out=sq[:], in_=xt[:, :, :, 0:C + 1], func=Act.Square)
    xbf = sqp.tile([P, KS, RPP, 128], bf16, tag="xbf")
    nc.vector.tensor_copy(out=xbf[:], in_=xt[:, :, :, 0:128])
    oh = ohp.tile([P, KS, RPP, GPC], bf16, tag="oh")
    bid_ap = xt[:, :, :, C + 1:C + 2]
    in0 = bass.AP(tensor=bid_ap.tensor, offset=bid_ap.offset,
                  ap=[bid_ap.ap[0], bid_ap.ap[1], bid_ap.ap[2], [0, GPC]])
    it = cs["iota_t"][:]
    in1 = bass.AP(tensor=it.tensor, offset=it.offset,
                  ap=[it.ap[0], [0, KS], [0, RPP], it.ap[1]])
    nc.vector.tensor_tensor(out=oh[:], in0=in0, in1=in1, op=Alu.is_equal)
    return sq, xbf, oh


def _params_ops(nc, bass, mybir, cp, tag, acc_sq, acc_x, cs):
    """(Re)compute affine params from accumulated sums; returns fp16 hi/lo.

    Entries for incomplete graphs are garbage but kept finite (counts
    clamped >= 1, variance clamped >= 0); ready rows never reference them.
    """
    f32 = mybir.dt.float32
    bf16 = mybir.dt.float16
    Alu = mybir.AluOpType
    Act = mybir.ActivationFunctionType
    t = lambda shape, dt, name: cp.tile(shape, dt, tag=name, name=name)

    invc = t([GPC, 1], f32, "invc")
    nc.vector.tensor_scalar_max(out=invc[:], in0=acc_sq[:, C:C + 1],
                                scalar1=1.0)
    nc.vector.reciprocal(out=invc[:], in_=invc[:])
    esq = t([GPC, C], f32, "esq")
    nc.vector.tensor_scalar_mul(out=esq[:], in0=acc_sq[:, 0:C],
                                scalar1=invc[:])
    m_t = t([GPC, 128], f32, "m_t")
    nc.vector.tensor_scalar_mul(out=m_t[:], in0=acc_x[:], scalar1=invc[:])

    var = t([GPC, 128], f32, "var")
    nc.vector.tensor_tensor(out=var[:], in0=m_t[:], in1=m_t[:], op=Alu.mult)
    nc.vector.tensor_tensor(out=var[:], in0=esq[:, 0:128], in1=var[:],
                            op=Alu.subtract)
    nc.vector.tensor_scalar_max(out=var[:], in0=var[:], scalar1=0.0)
    e3 = t([GPC, 64], f32, "e3")
    nc.vector.tensor_reduce(out=e3[:],
                            in_=esq[:, 128:320].rearrange(
                                "p (j d) -> p j d", d=3),
                            axis=mybir.AxisListType.X, op=Alu.add)
    e5 = t([GPC, 32], f32, "e5")
    nc.vector.tensor_reduce(out=e5[:],
                            in_=esq[:, 320:480].rearrange(
                                "p (j d) -> p j d", d=5),
                            axis=mybir.AxisListType.X, op=Alu.add)

    # rstd = 1/sqrt(fn + eps); Rsqrt on ACT is banned for accuracy
    eps_t = cs["eps_t"]
    nc.scalar.activation(out=var[:], in_=var[:], func=Act.Sqrt,
                         bias=eps_t[:], scale=1.0)
    nc.vector.reciprocal(out=var[:], in_=var[:])
    nc.scalar.activation(out=e3[:], in_=e3[:], func=Act.Sqrt,
                         bias=eps_t[:], scale=1.0 / 3.0)
    nc.vector.reciprocal(out=e3[:], in_=e3[:])
    nc.scalar.activation(out=e5[:], in_=e5[:], func=Act.Sqrt,
                         bias=eps_t[:], scale=1.0 / 5.0)
    nc.vector.reciprocal(out=e5[:], in_=e5[:])

    # params: [0:128]=A_s, [128:256]=B_s, [256:320]=A_3, [320:352]=A_5
    params = t([GPC, 352], f32, "params")
    nc.vector.tensor_tensor(out=params[:, 0:128], in0=var[:],
                            in1=cs["w_b"][:, 0:128], op=Alu.mult)
    bm = t([GPC, 128], f32, "bm")
    nc.vector.tensor_tensor(out=bm[:], in0=m_t[:], in1=params[:, 0:128],
                            op=Alu.mult)
    nc.vector.tensor_tensor(out=params[:, 128:256], in0=cs["bias_b"][:],
                            in1=bm[:], op=Alu.subtract)
    nc.vector.tensor_tensor(out=params[:, 256:320], in0=e3[:],
                            in1=cs["w_b"][:, 128:192], op=Alu.mult)
    nc.vector.tensor_tensor(out=params[:, 320:352], in0=e5[:],
                            in1=cs["w_b"][:, 192:224], op=Alu.mult)

    par_h = cp.tile([GPC, 352], bf16, tag=f"par_h{tag}")
    nc.vector.tensor_copy(out=par_h[:], in_=params[:])
    ph32 = t([GPC, 352], f32, "ph32")
    nc.vector.tensor_copy(out=ph32[:], in_=par_h[:])
    par_l = cp.tile([GPC, 352], bf16, tag=f"par_l{tag}")
    nc.vector.tensor_tensor(out=par_l[:], in0=params[:], in1=ph32[:],
                            op=Alu.subtract)
    return par_h, par_l


def _apply_ops(nc, bass, mybir, ohp, ps2, out_g, g, xt, par_h, par_l, cs):
    """Gather params for group g's rows and normalize in place, then store."""
    f32 = mybir.dt.float32
    bf16 = mybir.dt.float16
    Alu = mybir.AluOpType
    bt = ohp.tile([GPC, GROUP], f32, tag="bt")
    nc.gpsimd.dma_start(out=bt[:], in_=bass.AP(
        tensor=nc.t_bid, offset=g * GROUP, ap=[[0, GPC], [1, GROUP]]))
    ohT = ohp.tile([GPC, GROUP], bf16, tag="ohT")
    nc.vector.tensor_scalar(out=ohT[:], in0=bt[:], scalar1=cs["iota_c"][:],
                            scalar2=None, op0=Alu.is_equal)

    Act = mybir.ActivationFunctionType
    for k in range(KS):
        for h in range(RPP // HGRP):
            gp = ps2.tile([P, HGRP, 512], f32, tag="gp")
            for rr in range(HGRP):
                r = h * HGRP + rr
                # subtile (k, r) rows sit at k*P*RPP + r, stride RPP
                o = ohT[:]
                lhsT = bass.AP(tensor=o.tensor,
                               offset=o.offset + k * P * RPP + r,
                               ap=[o.ap[0], [RPP, P]])
                nc.tensor.matmul(out=gp[:, rr, 0:352], lhsT=lhsT,
                                 rhs=par_h[:], start=True, stop=False)
                nc.tensor.matmul(out=gp[:, rr, 0:352], lhsT=lhsT,
                                 rhs=par_l[:], start=False, stop=True)
            # stage gathered params in SBUF via ACT (it can read PSUM) so
            # the elementwise work can spread over DVE + GpSimd
            gpsb = ohp.tile([P, HGRP, 352], f32, tag="gpsb")
            nc.scalar.activation(out=gpsb[:], in_=gp[:, :, 0:352],
                                 func=Act.Copy)
            rs = slice(h * HGRP, (h + 1) * HGRP)
            s0 = xt[:, k, rs, 0:128]
            nc.vector.tensor_tensor(out=s0, in0=s0, in1=gpsb[:, :, 0:128],
                                    op=Alu.mult)
            nc.vector.tensor_tensor(out=s0, in0=s0, in1=gpsb[:, :, 128:256],
                                    op=Alu.add)
            s3 = xt[:, k, rs, 128:320].rearrange("p k (j d) -> p k j d", d=3)
            nc.gpsimd.tensor_tensor(out=s3, in0=s3,
                                    in1=_expand(gpsb[:, :, 256:320], 3, bass),
                                    op=Alu.mult)
            s5 = xt[:, k, rs, 320:480].rearrange("p k (j d) -> p k j d", d=5)
            nc.gpsimd.tensor_tensor(out=s5, in0=s5,
                                    in1=_expand(gpsb[:, :, 320:352], 5, bass),
                                    op=Alu.mult)
    # keep the Sync HWDGE ring dedicated to loads: a store queued behind a
    # load's slot-release wait would stall the DMA drain (FIFO per ring)
    for k in range(KS):
        nc.scalar.dma_start(out=out_g[g][:, k], in_=xt[:, k, :, 0:C])


def _build_stream(n_pad, fstar):
    """Single-pass sliding-window program."""
    import concourse.bacc as bacc
    import concourse.bass as bass
    import concourse.tile as tile
    from concourse import mybir

    f32 = mybir.dt.float32
    Alu = mybir.AluOpType
    ng = n_pad // GROUP
    applies_at = {}
    for t, f in enumerate(fstar):
        applies_at.setdefault(f, []).append(t)

    nc = bacc.Bacc("TRN2", target_bir_lowering=False, debug=False,
                   num_devices=NCORES)
    x_g, out_g = _declare_io(nc, mybir, n_pad)

    with tile.TileContext(nc) as tc:
        with (
            tc.tile_pool(name="const", bufs=1) as cp,
            tc.tile_pool(name="par", bufs=2) as pp,
            tc.tile_pool(name="xt", bufs=W_MAX) as xp,
            tc.tile_pool(name="sq", bufs=2) as sqp,
            tc.tile_pool(name="oh", bufs=2) as ohp,
            tc.tile_pool(name="ps1", bufs=2, space="PSUM") as ps1,
            tc.tile_pool(name="ps2", bufs=2, space="PSUM") as ps2,
        ):
            cs = _setup(nc, bass, tile, mybir, tc, cp)
            acc_sq = cp.tile([GPC, C + 1], f32, tag="acc_sq")
            acc_x = cp.tile([GPC, 128], f32, tag="acc_x")
            nc.vector.memset(acc_sq[:], 0.0)
            nc.vector.memset(acc_x[:], 0.0)

            xts = {}
            for t in range(ng):
                xt = xp.tile([P, KS, RPP, CW], f32, tag="xa")
                xts[t] = xt
                nc.sync.dma_start(out=xt[:], in_=x_g[t])
                sq, xbf, oh = _phase1_ops(nc, bass, mybir, sqp, ohp, xt, cs)
                p_sq = ps1.tile([GPC, C + 1], f32, tag="p_sq")
                p_x = ps1.tile([GPC, 128], f32, tag="p_x")
                for k in range(KS):
                    for r in range(RPP):
                        st = (k == 0 and r == 0)
                        sp = (k == KS - 1 and r == RPP - 1)
                        lhsT = oh[:, k, r, :]
                        nc.tensor.matmul(out=p_sq[:], lhsT=lhsT,
                                         rhs=sq[:, k, r, :],
                                         start=st, stop=sp)
                        nc.tensor.matmul(out=p_x[:], lhsT=lhsT,
                                         rhs=xbf[:, k, r, :],
                                         start=st, stop=sp)
                nc.vector.tensor_tensor(out=acc_sq[:], in0=acc_sq[:],
                                        in1=p_sq[:], op=Alu.add)
                nc.vector.tensor_tensor(out=acc_x[:], in0=acc_x[:],
                                        in1=p_x[:], op=Alu.add)

                if t in applies_at:
                    par_h, par_l = _params_ops(nc, bass, mybir, pp, "",
                                               acc_sq, acc_x, cs)
                    for tp in applies_at[t]:
                        _apply_ops(nc, bass, mybir, ohp, ps2, out_g, tp,
                                   xts.pop(tp), par_h, par_l, cs)

    nc.compile()
    return nc


def _build_twopass(n_pad):
    """Fallback: stats pass + re-read apply pass (bounded SBUF window)."""
    import concourse.bacc as bacc
    import concourse.bass as bass
    import concourse.tile as tile
    from concourse import mybir

    f32 = mybir.dt.float32
    Alu = mybir.AluOpType
    ng = n_pad // GROUP
    r_cache = min(R_CACHE, ng)

    nc = bacc.Bacc("TRN2", target_bir_lowering=False, debug=False,
                   num_devices=NCORES)
    x_g, out_g = _declare_io(nc, mybir, n_pad)

    with tile.TileContext(nc) as tc:
        with (
            tc.tile_pool(name="const", bufs=1) as cp,
            tc.tile_pool(name="par", bufs=1) as pp,
            tc.tile_pool(name="xt", bufs=XT_BUFS) as xp,
            tc.tile_pool(name="xcache", bufs=max(r_cache, 1)) as xcp,
            tc.tile_pool(name="sq", bufs=2) as sqp,
            tc.tile_pool(name="oh", bufs=2) as ohp,
            tc.tile_pool(name="ps1", bufs=2, space="PSUM") as ps1,
            tc.tile_pool(name="ps2", bufs=2, space="PSUM") as ps2,
        ):
            cs = _setup(nc, bass, tile, mybir, tc, cp)
            acc_sq = cp.tile([GPC, C + 1], f32, tag="acc_sq")
            acc_x = cp.tile([GPC, 128], f32, tag="acc_x")
            nc.vector.memset(acc_sq[:], 0.0)
            nc.vector.memset(acc_x[:], 0.0)

            cached = {}
            for g in range(ng):
                if g >= ng - r_cache:
                    xt = xcp.tile([P, KS, RPP, CW], f32, tag="xc")
                    cached[g] = xt
                else:
                    xt = xp.tile([P, KS, RPP, CW], f32, tag="xa")
                nc.sync.dma_start(out=xt[:], in_=x_g[g])
                sq, xbf, oh = _phase1_ops(nc, bass, mybir, sqp, ohp, xt, cs)
                p_sq = ps1.tile([GPC, C + 1], f32, tag="p_sq")
                p_x = ps1.tile([GPC, 128], f32, tag="p_x")
                for k in range(KS):
                    for r in range(RPP):
                        st = (k == 0 and r == 0)
                        sp = (k == KS - 1 and r == RPP - 1)
                        lhsT = oh[:, k, r, :]
                        nc.tensor.matmul(out=p_sq[:], lhsT=lhsT,
                                         rhs=sq[:, k, r, :],
                                         start=st, stop=sp)
                        nc.tensor.matmul(out=p_x[:], lhsT=lhsT,
                                         rhs=xbf[:, k, r, :],
                                         start=st, stop=sp)
                nc.vector.tensor_tensor(out=acc_sq[:], in0=acc_sq[:],
                                        in1=p_sq[:], op=Alu.add)
                nc.vector.tensor_tensor(out=acc_x[:], in0=acc_x[:],
                                        in1=p_x[:], op=Alu.add)

            par_h, par_l = _params_ops(nc, bass, mybir, pp, "", acc_sq,
                                       acc_x, cs)
            for g in range(ng):
                if g in cached:
                    xt = cached[g]
                else:
                    xt = xp.tile([P, KS, RPP, CW], f32, tag="xa")
                    nc.sync.dma_start(out=xt[:], in_=x_g[g])
                _apply_ops(nc, bass, mybir, ohp, ps2, out_g, g, xt,
                           par_h, par_l, cs)

    nc.compile()
    return nc


def kernel(input, batch_id_tensor, weight, bias, _trace=False):
    from concourse import bass_utils

    x = np.ascontiguousarray(np.asarray(input, dtype=np.float32))
    bid = np.asarray(batch_id_tensor).astype(np.int64)
    w = np.asarray(weight, dtype=np.float32)
    b = np.asarray(bias, dtype=np.float32)
    n = x.shape[0]

    # graph-aligned core boundaries
    edges = np.searchsorted(bid, np.arange(0, G + 1, GPC), side="left")
    rows = np.diff(edges)
    n_pad = max(GROUP, int(-(-rows.max() // GROUP)) * GROUP)
    ng = n_pad // GROUP

    # f*(t): first group index by which every graph touching group t is
    # fully consumed, maxed across cores so one SPMD program serves all
    fstar = np.arange(ng)
    for c in range(NCORES):
        lo, hi = int(edges[c]), int(edges[c + 1])
        lb = bid[lo:hi] - c * GPC
        ge = np.searchsorted(lb, np.arange(GPC + 1))
        nrows = hi - lo
        for t in range(ng):
            last = min((t + 1) * GROUP, nrows) - 1
            if last < t * GROUP:
                continue
            gl = int(np.searchsorted(ge, last, side="right")) - 1
            end_row = int(ge[gl + 1]) - 1 if gl + 1 <= GPC else nrows - 1
            fstar[t] = max(fstar[t], end_row // GROUP)
    w_need = int((fstar - np.arange(ng)).max()) + 1

    if w_need + 1 <= W_MAX:
        key = (n_pad, tuple(int(f) for f in fstar))
        if key not in _prog_cache:
            _prog_cache[key] = _build_stream(n_pad, tuple(fstar))
    else:
        key = (n_pad, None)
        if key not in _prog_cache:
            _prog_cache[key] = _build_twopass(n_pad)
    nc = _prog_cache[key]

    iota = np.arange(GPC, dtype=np.float32)
    in_maps = []
    for c in range(NCORES):
        lo, hi = int(edges[c]), int(edges[c + 1])
        nc_rows = hi - lo
        xa = np.empty((n_pad, CW), dtype=np.float32)
        xa[:nc_rows, 0:C] = x[lo:hi]
        xa[:nc_rows, C] = 1.0
        xa[:nc_rows, C + 1] = (bid[lo:hi] - c * GPC).astype(np.float32)
        if nc_rows < n_pad:
            xa[nc_rows:, 0:C] = 0.0
            xa[nc_rows:, C] = 0.0
            xa[nc_rows:, C + 1] = GPC  # out-of-range id -> no one-hot match
        in_maps.append({
            "x": xa,
            "bid": np.ascontiguousarray(xa[:, C + 1]),
            "iota8": iota,
            "w": w,
            "b": b,
        })

    res = bass_utils.run_bass_kernel_spmd(
        nc, in_maps, core_ids=list(range(NCORES)), trace=_trace)

    out = np.empty((n, C), dtype=np.float32)
    for c in range(NCORES):
        lo, hi = int(edges[c]), int(edges[c + 1])
        out[lo:hi] = res.results[c]["out"][:hi - lo]
    if _trace:
        return out, res
    return out
